# revision 23
# baseline (speedup 1.0000x reference)
"""Distributed Trainium2 kernel for nn_Attention (B=2,S=4096,D=2048,H=16).

Tensor-parallel over heads across 8 NeuronCores; core c owns heads 2c,2c+1.

Host prep (free): x -> xT [D, B*S] fp16; per-core wq/wk/wv column slices
pre-transposed, with rotary pair de-interleave folded into the wq/wk row
permutation; rotary cos/sin combined with the RMS-norm weights into 4
coefficient planes; wo pre-transposed.

Per core:
  1. QK over 512-wide s-chunks (x DMA split across both HWDGE queues,
     triple-buffered, so the PE never waits and HAM stays warm). RMS-norm
     partition-reduction via a ones matmul into a [128,512] broadcast
     layout; 1/sqrt(var) = ACT Sqrt + DVE reciprocal_approx_fast (Square
     and Sqrt share one ACT table set, so phase 1 pays a single table
     load instead of 128 ln/exp set reloads); rotary on VectorE. v is
     computed directly in natural [s, hd] layout (x chunk stationary,
     wv both heads streaming N=256, 2-per-bank PSUM accumulators) so no
     PE transposes are needed; a ones column at hd is appended for the
     PV row sums. Epilogues software-pipelined one matmul group behind.
  2. Attention per (b, head, 512-wide q block): scoresT = kT.T @ qT with
     N=512 matmuls (LDWEIGHTS fully hidden), exp on ScalarE on [128,2,512]
     score pairs straight out of PSUM (bf16 out; scores bounded so no
     max-subtraction), PV accumulates probsT.T @ [v|1] into a [128,4,256]
     PSUM tile (two 129-wide accumulators per bank; start=True only on the
     bank-leading accumulator since it clears has_written bank-wide)
     giving attention output and softmax row sums in one pass. PV emitted
     2 exp-pairs behind QK; the epilogue's PE transposes are deferred one
     block so the DVE reciprocal chain never stalls the PE.
  3. Output ownership is stride-4 interleaved: core j owns q-tiles
     {4k + j%4} of batch j//4. Each of 4 passes computes q-blocks 2t,2t+1
     for every (b,h), AllToAll's one 256-col chunk per dest, and the output
     projection for the previous pass runs behind it with wo streamed from
     DRAM in 2MB pieces on alternating queues. The last pass ships its two
     128-col halves separately and interleaves its projection so only the
     st=1 pieces sit behind the final AllToAll's latency.
Host reassembles the interleaved row blocks.
"""
import sys

sys.path.insert(0, "/opt/trn_rl_repo")

import numpy as np
import ml_dtypes

import concourse.bass as bass
import concourse.bacc as bacc
import concourse.mybir as mybir
import concourse.tile as tile
from concourse import masks
from concourse.bass_utils import run_bass_kernel_spmd

DT16 = mybir.dt.float16
BF16 = mybir.dt.bfloat16
F32 = mybir.dt.float32

B, S, D, H = 2, 4096, 2048, 16
HD = 128                  # head dim
NCORES = 8
HPC = H // NCORES         # heads per core = 2
BS = B * S                # 8192
KC = D // 128             # 16 contraction chunks
SCH = 512                 # s-chunk for QKV phase
NSCH = BS // SCH          # 16
SLICE = BS // NCORES      # 1024 output rows per core
NT = 4                    # attention/a2a passes
EPS = 1e-5
ISQ = 1.0 / np.sqrt(HD)

_CACHE = {}


def _build():
    nc = bacc.Bacc("TRN2", target_bir_lowering=False, debug=False,
                   num_devices=NCORES)

    xt = nc.dram_tensor("xt", [D, BS], DT16, kind="ExternalInput")
    wqt = nc.dram_tensor("wqt", [D, HPC * HD], DT16, kind="ExternalInput")
    wkt = nc.dram_tensor("wkt", [D, HPC * HD], DT16, kind="ExternalInput")
    wvt = nc.dram_tensor("wvt", [D, HPC * HD], DT16, kind="ExternalInput")
    wot = nc.dram_tensor("wot", [D, D], DT16, kind="ExternalInput")
    # plane 0 rows = [A(64); B(64)], plane 1 rows = [C(64); D(64)] so every
    # rotary multiply pairs SBUF operands with equal base partition.
    rq = nc.dram_tensor("rq", [2, 128, BS], DT16, kind="ExternalInput")
    rk = nc.dram_tensor("rk", [2, 128, BS], DT16, kind="ExternalInput")
    out_ext = nc.dram_tensor("out", [SLICE, D], F32, kind="ExternalOutput")

    with tile.TileContext(nc) as tc:
        with tc.tile_pool(name="persist", bufs=1) as pp, \
             tc.tile_pool(name="dramp", bufs=1, space="DRAM") as dramp:
            ident = pp.tile([128, 128], DT16)
            masks.make_identity(nc, ident[:])
            ones_sq = pp.tile([128, 128], DT16)
            nc.gpsimd.memset(ones_sq[:], 1.0)
            eps_t = pp.tile([128, 1], F32)
            nc.gpsimd.memset(eps_t[:], EPS)

            # per-head tensors living through phases 1-2
            qkvp = tc.alloc_tile_pool(name="qkvp", bufs=1)
            q_sb = [qkvp.tile([128, BS], DT16, name=f"q{h}")
                    for h in range(HPC)]
            k_sb = [qkvp.tile([128, BS], DT16, name=f"k{h}")
                    for h in range(HPC)]
            # v in natural layout per 128-row s-tile, ones column at 128
            v_sb = [qkvp.tile([128, BS // 128, HD + 1], BF16, name=f"v{h}")
                    for h in range(HPC)]
            for h in range(HPC):
                nc.gpsimd.memset(v_sb[h][:, :, HD:HD + 1], 1.0)

            # ---------------- Phase 1: QKV + RMS + rotary ----------------
            with tc.tile_pool(name="p1", bufs=1) as p1, \
                 tc.tile_pool(name="p1ps", bufs=1,
                              space=bass.MemorySpace.PSUM) as p1ps:
                wq_s = p1.tile([128, KC, HPC * HD], DT16)
                wk_s = p1.tile([128, KC, HPC * HD], DT16)
                wv_s = p1.tile([128, KC, HPC * HD], DT16)

                def ep_qk(kind, h, ps, rt, s0):
                    dst = (q_sb if kind == "q" else k_sb)[h]
                    # stage ps into SBUF with an ACT Copy (Copy is filler in
                    # every table set, unlike Square which lives in a
                    # different set than Sqrt and forced a 1.3us table
                    # reload per epilogue); the square then runs on the DVE
                    # (it cannot read two PSUM operands) and the qn multiply
                    # reuses the staged copy, releasing the PSUM buffer early
                    qc = p1.tile([128, SCH], DT16, tag="qc", bufs=3,
                                 name="qc")
                    nc.scalar.copy(qc[:], ps[:])
                    sq = p1.tile([128, SCH], DT16, tag="sqv", bufs=3,
                                 name="sq")
                    nc.vector.tensor_mul(sq[:], qc[:], qc[:])
                    ssum = p1ps.tile([128, SCH], F32, tag="ssum", bufs=1,
                                     name="ssum")
                    nc.tensor.matmul(ssum[:], ones_sq[:], sq[:],
                                     start=True, stop=True)
                    # 1/sqrt(var+eps) via ACT Sqrt + DVE fast reciprocal.
                    # Square+Sqrt share one ACT table set, so phase 1 pays a
                    # single ACT_TABLE_LOAD instead of thrashing ln/exp sets
                    # (the ln/exp pair cost 128 reloads = 165us of ScalarE).
                    std = p1.tile([128, SCH], F32, tag="sqv", bufs=3,
                                  name="std")
                    nc.scalar.activation(
                        std[:], ssum[:], mybir.ActivationFunctionType.Sqrt,
                        bias=eps_t[:], scale=1.0 / HD)
                    rstd = p1.tile([128, SCH], F32, tag="sqv", bufs=3,
                                   name="rstd")
                    nc.vector.reciprocal_approx_fast(rstd[:], std[:])
                    qn = p1.tile([128, SCH], DT16, tag="qn", bufs=2,
                                 name="qn")
                    nc.vector.tensor_mul(qn[:], qc[:], rstd[:])
                    xr, xi = qn[0:64, :], qn[64:128, :]
                    ta = p1.tile([64, SCH], DT16, tag="rot0", bufs=2,
                                 name="ta")
                    tb = p1.tile([64, SCH], DT16, tag="rot1", bufs=2,
                                 name="tb")
                    nc.vector.tensor_mul(ta[:], xr, rt[0:64, 0, :])
                    nc.vector.tensor_mul(tb[:], xi, rt[64:128, 0, :])
                    nc.vector.tensor_sub(dst[0:64, s0:s0 + SCH],
                                         ta[:], tb[:])
                    tc2 = p1.tile([64, SCH], DT16, tag="rot0", bufs=2,
                                  name="tc2")
                    td = p1.tile([64, SCH], DT16, tag="rot1", bufs=2,
                                  name="td")
                    nc.vector.tensor_mul(tc2[:], xr, rt[0:64, 1, :])
                    nc.vector.tensor_mul(td[:], xi, rt[64:128, 1, :])
                    nc.vector.tensor_add(dst[64:128, s0:s0 + SCH],
                                         tc2[:], td[:])

                def p1_epilogue(kind, h, ps, rt, sc):
                    ep_qk(kind, h, ps, rt, sc * SCH)

                # warm the HAM clock gate during the startup DMA wait:
                # dependency-free matmuls on the identity keep the PE busy
                # (transpose-mode would not count) so chunk 0 runs at full
                # clock instead of K=4/8
                for _ in range(72):
                    wps = p1ps.tile([128, 128], F32, tag="mm", bufs=5)
                    nc.tensor.matmul(wps[:], ident[:], ident[:],
                                     start=True, stop=True)

                # 2-deep software pipeline: each output's epilogue is
                # emitted two matmul groups later, giving ScalarE/DVE a
                # full group of slack before the PE consumes their output
                pend = []
                xr_ap = xt.ap().rearrange("(kc p) s -> p kc s", p=128)
                rq_ap = rq.ap().rearrange("f p s -> p f s")
                rk_ap = rk.ap().rearrange("f p s -> p f s")
                for sc in range(NSCH):
                    s0 = sc * SCH
                    # split the 2MB x chunk across both HWDGE queues
                    # (SP + Act) and triple-buffer so the PE never waits
                    # (chunk 0's x goes out in interleaved quarters below)
                    xt_t = p1.tile([128, KC, SCH], DT16, tag="xt", bufs=3)
                    if sc > 0:
                        nc.sync.dma_start(xt_t[:, 0:8, :],
                                          xr_ap[:, 0:8, s0:s0 + SCH])
                        nc.scalar.dma_start(xt_t[:, 8:16, :],
                                            xr_ap[:, 8:16, s0:s0 + SCH])
                    rq_t = p1.tile([128, 2, SCH], DT16, tag="rq", bufs=2)
                    rk_t = p1.tile([128, 2, SCH], DT16, tag="rk", bufs=2)
                    if sc == 0:
                        # startup: interleave kc-quarters of x and wq so
                        # the first matmul group starts as soon as the
                        # kc 0-3 slices land (subtile deps); rotary and
                        # wk/wv stream in behind while the q groups run
                        wr_q = wqt.ap().rearrange("(kc p) m -> p kc m",
                                                  p=128)
                        for q0 in (0, 8):
                            nc.sync.dma_start(
                                xt_t[:, q0:q0 + 4, :],
                                xr_ap[:, q0:q0 + 4, s0:s0 + SCH])
                            nc.scalar.dma_start(
                                xt_t[:, q0 + 4:q0 + 8, :],
                                xr_ap[:, q0 + 4:q0 + 8, s0:s0 + SCH])
                            nc.sync.dma_start(wq_s[:, q0:q0 + 4, :],
                                              wr_q[:, q0:q0 + 4, :])
                            nc.scalar.dma_start(wq_s[:, q0 + 4:q0 + 8, :],
                                                wr_q[:, q0 + 4:q0 + 8, :])
                        for wsb_, wsrc in ((None, None),
                                           (wk_s, wkt), (wv_s, wvt)):
                            if wsb_ is None:
                                nc.sync.dma_start(
                                    rq_t[:], rq_ap[:, :, s0:s0 + SCH])
                                nc.scalar.dma_start(
                                    rk_t[:], rk_ap[:, :, s0:s0 + SCH])
                                continue
                            wr_ = wsrc.ap().rearrange("(kc p) m -> p kc m",
                                                      p=128)
                            nc.sync.dma_start(wsb_[:, 0:8, :],
                                              wr_[:, 0:8, :])
                            nc.scalar.dma_start(wsb_[:, 8:16, :],
                                                wr_[:, 8:16, :])
                    else:
                        nc.sync.dma_start(rq_t[:],
                                          rq_ap[:, :, s0:s0 + SCH])
                        nc.scalar.dma_start(rk_t[:],
                                            rk_ap[:, :, s0:s0 + SCH])

                    if sc == 0:
                        # kind-major: chunk 0 so q groups run while wk/wv
                        # stream in
                        order = [(h, kind) for kind in ("q", "k")
                                 for h in range(HPC)]
                    else:
                        order = [(h, kind) for h in range(HPC)
                                 for kind in ("q", "k")]
                    for h, kind in order:
                        hs = h * HD
                        wsb = wq_s if kind == "q" else wk_s
                        rt = rq_t if kind == "q" else rk_t
                        ps = p1ps.tile([128, SCH], F32, tag="mm",
                                       bufs=5)
                        for kc in range(KC):
                            nc.tensor.matmul(
                                ps[:], wsb[:, kc, hs:hs + HD],
                                xt_t[:, kc, :],
                                start=(kc == 0), stop=(kc == KC - 1))
                        pend.append((kind, h, ps, rt, sc))
                        # shallower pipeline near the end so the final DVE
                        # chains spread out instead of bunching after the
                        # last chunk's matmuls
                        depth = 2 if sc < NSCH - 2 else 1
                        if len(pend) > depth:
                            p1_epilogue(*pend.pop(0))
                    if sc == NSCH - 1:
                        # drain the q/k epilogues before the v group so the
                        # final chains overlap the v matmuls instead of
                        # serializing with phase 2's first QK
                        while pend:
                            p1_epilogue(*pend.pop(0))
                    # v computed directly in natural [s, hd] layout: the x
                    # chunk slice is the stationary operand, wv (both heads,
                    # N=256) streams.  Replaces the old v^T matmuls + 4 PE
                    # transposes + DVE staging copies per chunk.  4 psum
                    # accumulators pack 2 per bank: start=True only on the
                    # bank-leading accumulator (it clears has_written
                    # bank-wide).
                    vacc = p1ps.tile([128, 4, HPC * HD], F32, tag="vacc",
                                     bufs=1, name="vacc")
                    for kc in range(KC):
                        for st4 in range(4):
                            nc.tensor.matmul(
                                vacc[:, st4, :],
                                xt_t[:, kc, st4 * 128:(st4 + 1) * 128],
                                wv_s[:, kc, :],
                                start=(kc == 0 and st4 % 2 == 0),
                                stop=(kc == KC - 1),
                                skip_group_check=True)
                    # drain one q/k epilogue behind the v matmul group
                    if pend:
                        p1_epilogue(*pend.pop(0))
                    for st4 in range(4):
                        for h in range(HPC):
                            nc.vector.tensor_copy(
                                v_sb[h][:, sc * 4 + st4, 0:HD],
                                vacc[:, st4, h * HD:(h + 1) * HD])
                for item in pend:
                    p1_epilogue(*item)
                # preload the exp ACT table set during the phase boundary so
                # the first attention block's exp doesn't pay the ~2.7us
                # table load on the critical path
                dume = p1.tile([128, 1], DT16, name="dume")
                nc.scalar.activation(dume[:], eps_t[:],
                                     mybir.ActivationFunctionType.Exp)

            # ---------------- Phase 2: attention ----------------
            # 4 passes; pass t computes q-blocks m=2t,2t+1 (512 wide) for
            # every (b,h). Output ownership is stride-4 interleaved so each
            # 512-block contributes one 128-tile to every dest core of its
            # batch; after each pass an AllToAll ships a [2048,256] chunk
            # and the previous pass's output projection runs behind it.
            # last pass ships its two 128-col halves separately so the
            # final AllToAll overlaps the last blocks' compute
            a2a_in_t = [dramp.tile([D, 256], DT16, name=f"a2a_in{t}")
                        for t in range(NT - 1)]
            a2a_out_t = [dramp.tile([D, 256], DT16, name=f"a2a_out{t}")
                         for t in range(NT - 1)]
            a2a_in_h = [dramp.tile([D, 128], DT16, name=f"a2a_inh{u}")
                        for u in range(2)]
            a2a_out_h = [dramp.tile([D, 128], DT16, name=f"a2a_outh{u}")
                         for u in range(2)]
            with tc.tile_pool(name="p2", bufs=1) as p2, \
                 tc.tile_pool(name="p3", bufs=1) as p3, \
                 tc.tile_pool(name="p2ps", bufs=1,
                              space=bass.MemorySpace.PSUM) as p2ps:
                wo_ap = wot.ap().rearrange("(kc p) m -> p kc m", p=128)
                wo_tiles = {}

                def emit_wo_dma(t, n):
                    wt = p3.tile([128, KC, 512], DT16, tag="wo", bufs=3,
                                 name="wo_t")
                    eng = nc.sync if n % 2 == 0 else nc.scalar
                    eng.dma_start(wt[:],
                                  wo_ap[:, :, n * 512:(n + 1) * 512])
                    wo_tiles[(t, n)] = wt

                pend_tr = []

                def flush_tr():
                    # PE part of the previous block's epilogue (transposes
                    # + staging copies), deferred one block so the DVE/Act
                    # normalization chain never stalls the PE
                    while pend_tr:
                        att, att_c, b, h, sub, mpar = pend_tr.pop(0)
                        tp2 = p2ps.tile([128, 128], DT16, tag="aux", bufs=2,
                                        name="tp2")
                        nc.tensor.transpose(tp2[:], att[:], ident[:])
                        nc.vector.tensor_copy(
                            att_c[:, b, h, sub, mpar, :], tp2[:])

                def attention_block512(b, h, m, att_c, mpar):
                    qc = b * S + m * 512
                    # 4 PV accumulators packed 2 per PSUM bank
                    ops = p2ps.tile([128, 4, 256], F32, tag="ops", bufs=1,
                                    name="ops")
                    def emit_pv(pb, kq2):
                        for i in range(2):
                            jt = b * 32 + kq2 * 2 + i
                            for sub in range(4):
                                # start=True clears has_written for the
                                # WHOLE bank, so with 2 accumulators per
                                # bank only the bank-leading sub (0, 2) of
                                # the very first matmul may set it; the
                                # other accumulators' first write lands on
                                # cleared bits and overwrites stale data.
                                nc.tensor.matmul(
                                    ops[:, sub, 0:HD + 1],
                                    pb[:, i, sub * 128:(sub + 1) * 128],
                                    v_sb[h][:, jt, :],
                                    start=(kq2 == 0 and i == 0
                                           and sub % 2 == 0),
                                    stop=(kq2 == 15 and i == 1),
                                    skip_group_check=True)

                    # 2-deep pipeline: PV for pair kq2-2 emitted after
                    # QK/exp of kq2 so ScalarE has slack before PE consumes
                    pending = []
                    for kq2 in range(16):
                        scs = p2ps.tile([128, 2, 512], F32, tag="scs",
                                        bufs=2, name="scs")
                        for i in range(2):
                            kc0 = b * S + (kq2 * 2 + i) * 128
                            nc.tensor.matmul(
                                scs[:, i, :],
                                k_sb[h][:, kc0:kc0 + 128],
                                q_sb[h][:, qc:qc + 512],
                                start=True, stop=True)
                        pb = p2.tile([128, 2, 512], BF16, tag="pb", bufs=4,
                                     name="pb")
                        nc.scalar.activation(
                            pb[:], scs[:],
                            mybir.ActivationFunctionType.Exp, scale=ISQ)
                        pending.append((pb, kq2))
                        if len(pending) > 2:
                            emit_pv(*pending.pop(0))
                        if kq2 == 6:
                            flush_tr()
                    for item in pending:
                        emit_pv(*item)
                    # epilogue: row sums sit at [:, sub, 128]; reciprocal
                    # on DVE straight out of PSUM (ScalarE is the binding
                    # engine in this phase; the deferred transposes absorb
                    # the chain latency)
                    rs = p2.tile([128, 4], F32, tag="rs", bufs=2, name="rs")
                    nc.vector.reciprocal(rs[:], ops[:, :, HD:HD + 1])
                    for sub in range(4):
                        att = p2.tile([128, 128], DT16, tag="att", bufs=8,
                                      name="att")
                        nc.vector.tensor_scalar_mul(
                            att[:], ops[:, sub, 0:HD], rs[:, sub:sub + 1])
                        pend_tr.append((att, att_c, b, h, sub, mpar))

                at_s_tiles = {}

                def load_at_s(t):
                    # prefetch the projection input for pass t as soon as
                    # its AllToAll result can land (emitted early in pass
                    # t+1 so the sync queue isn't clogged by the scatter)
                    at_s = p3.tile([128, KC, 256], DT16, tag="at_s",
                                   bufs=2, name="at_s")
                    nc.sync.dma_start(
                        at_s[:],
                        a2a_out_t[t][:].rearrange(
                            "(kc p) s -> p kc s", p=128))
                    at_s_tiles[t] = (at_s,)

                def load_at_s_half(t, u, eng):
                    # bufs=1 is deliberate: the half-1 gather then carries a
                    # WAR dependency on the st=0 projection pieces (which
                    # read half-0 from the same slot), so the Tile scheduler
                    # cannot hoist the gather -- and its ~35us collective
                    # wait -- ahead of the wo loads those pieces need on the
                    # same DMA queue (measured: 37us PE stall otherwise)
                    ah = p3.tile([128, KC, 128], DT16, tag="at_sh",
                                 bufs=1, name="at_sh")
                    eng.dma_start(
                        ah[:],
                        a2a_out_h[u][:].rearrange(
                            "(kc p) s -> p kc s", p=128))
                    at_s_tiles.setdefault(t, []).append(ah)

                def proj_piece(t, n, st, ats, wt, out_eng=None):
                    row0 = t * 256 + st * 128
                    po = p2ps.tile([128, 512], F32, tag="aux",
                                   bufs=2, name="po")
                    for kc in range(KC):
                        if len(ats) == 1:
                            lhs = ats[0][:, kc, st * 128:(st + 1) * 128]
                        else:
                            lhs = ats[st][:, kc, :]
                        nc.tensor.matmul(
                            po[:], lhs, wt[:, kc, :],
                            start=(kc == 0), stop=(kc == KC - 1))
                    ob = p3.tile([128, 512], F32, tag="ob", bufs=2,
                                 name="ob")
                    nc.vector.tensor_copy(ob[:], po[:])
                    (out_eng or nc.sync).dma_start(
                        out_ext.ap()[row0:row0 + 128,
                                     n * 512:(n + 1) * 512], ob[:])

                def proj_run(t, nlist, fire=()):
                    # fire: wo-piece DMAs to emit now; their slot waits
                    # resolve via this proj's own PE progress, so no
                    # engine head-of-line blocks
                    for key in fire:
                        if key[0] < NT and key not in wo_tiles:
                            emit_wo_dma(*key)
                    ats = at_s_tiles[t]
                    for n in nlist:
                        wt = wo_tiles.pop((t, n))
                        for st in range(2):
                            proj_piece(t, n, st, ats, wt)
                    if nlist[-1] == 3:
                        at_s_tiles.pop(t)

                def scatter_rows(att_c, mpar, b, h):
                    # the 4 dest-core DMAs for one attention block's half
                    for g in range(4):
                        j = b * 4 + g
                        r0 = j * HPC * HD + h * HD
                        nc.sync.dma_start(
                            a2a_in_h[mpar][r0:r0 + HD, :],
                            att_c[:, b, h, g, mpar, :])

                def fire_a2a_half(mpar):
                    nc.gpsimd.collective_compute(
                        "AllToAll", mybir.AluOpType.bypass,
                        ins=[a2a_in_h[mpar][:].opt()],
                        outs=[a2a_out_h[mpar][:].opt()],
                        replica_groups=[list(range(NCORES))])

                def scatter_a2a(t, att_c, mpar=None):
                    # dest core j rows <- its q-tiles, both heads
                    for j in range(NCORES):
                        bb, g = j // 4, j % 4
                        for h in range(HPC):
                            r0 = j * HPC * HD + h * HD
                            if mpar is None:
                                nc.sync.dma_start(
                                    a2a_in_t[t][r0:r0 + HD, :],
                                    att_c[:, bb, h, g, :, :])
                            else:
                                nc.sync.dma_start(
                                    a2a_in_h[mpar][r0:r0 + HD, :],
                                    att_c[:, bb, h, g, mpar, :])
                    if mpar is None:
                        ins, outs = a2a_in_t[t], a2a_out_t[t]
                    else:
                        ins, outs = a2a_in_h[mpar], a2a_out_h[mpar]
                    nc.gpsimd.collective_compute(
                        "AllToAll", mybir.AluOpType.bypass,
                        ins=[ins[:].opt()], outs=[outs[:].opt()],
                        replica_groups=[list(range(NCORES))])

                for t in range(NT):
                    nblk = 0
                    last = t == NT - 1
                    for mpar_o in ((0, 1),) if not last else ((0,), (1,)):
                        att_c = p2.tile([128, B, HPC, 4, 2, 128], DT16,
                                        tag="attc", bufs=2, name="att_c")
                        for b in range(B):
                            for h in range(HPC):
                                for mpar in mpar_o:
                                    attention_block512(b, h, 2 * t + mpar,
                                                       att_c, mpar)
                                    nblk += 1
                                    if t == 0 and nblk == 5:
                                        emit_wo_dma(0, 0)
                                    elif t == 0 and nblk == 7:
                                        emit_wo_dma(0, 1)
                                    elif t > 0 and nblk == 1:
                                        load_at_s(t - 1)
                                    if last and mpar == 1 and nblk >= 6:
                                        # scatter the PREVIOUS block (its
                                        # transposes flushed inside this
                                        # one) so the final collective can
                                        # fire right after the last block
                                        pb_, ph_ = divmod(nblk - 6, HPC)
                                        scatter_rows(att_c, 1, pb_, ph_)
                        # a slice of the previous pass's projection before
                        # the epilogue flush: PE work that covers the last
                        # block's DVE normalization latency
                        if not last:
                            if t > 0:
                                proj_run(t - 1, [0],
                                         fire=((t - 1, 2), (t - 1, 3)))
                            flush_tr()
                            scatter_a2a(t, att_c)
                        elif mpar_o[0] == 0:
                            # proj(2,[0]) is deferred to the tail: it both
                            # fills the final collective's flight and lets
                            # the half-0 scatter (and every following
                            # block) start ~8.5us earlier
                            emit_wo_dma(t - 1, 2)
                            emit_wo_dma(t - 1, 3)
                            flush_tr()
                            scatter_a2a(t, att_c, mpar=0)
                            # NOTE: must stay on nc.sync — a DMA that waits
                            # on the collective from the scalar queue blocks
                            # the whole ScalarE instruction stream and
                            # starves the attention exps (measured: 47us PE
                            # stall + HAM rethrottle)
                            load_at_s_half(t, 0, nc.sync)
                        else:
                            # second half: its AllToAll is tail-critical:
                            # flush + scatter only the final block (earlier
                            # ones were scattered per-block above), fire the
                            # collective, then fill its ~45us flight with
                            # the remaining pass-2 pieces and the pass-3
                            # st=0 pieces (first half already landed)
                            flush_tr()
                            scatter_rows(att_c, 1, B - 1, HPC - 1)
                            fire_a2a_half(1)
                            proj_run(t - 1, [1], fire=((t, 0), (t, 1)))
                    if 0 < t < NT - 1:
                        proj_run(t - 1, [1, 2, 3], fire=((t, 0), (t, 1)))
                # ---- tail: all st=0 pieces (first a2a half) run inside the
                # final collective's flight; st=1 pieces follow it.  The
                # at_sh single-buffer WAR makes the half-1 gather depend on
                # every st=0 piece, so the scheduler must place the wo loads
                # those pieces need ahead of the gather in the DMA queues.
                # wo(3,0) is loaded twice (slot rotation would otherwise
                # make wo(3,3)'s load wait on the st=1 piece behind the
                # collective -- a dependency cycle).
                t3 = NT - 1
                ats3 = at_s_tiles[t3]
                proj_run(t3 - 1, [0])
                proj_run(t3 - 1, [2])
                proj_piece(t3, 0, 0, ats3, wo_tiles.pop((t3, 0)))
                proj_run(t3 - 1, [3])
                emit_wo_dma(t3, 2)
                proj_piece(t3, 1, 0, ats3, wo_tiles[(t3, 1)])
                emit_wo_dma(t3, 3)
                # (3,3) before (3,2): their wo loads land in slot order
                # A((3,0)-freed-early) then C((2,3)-freed-late), so this
                # run order gives both transfers slack
                proj_piece(t3, 3, 0, ats3, wo_tiles[(t3, 3)])
                proj_piece(t3, 2, 0, ats3, wo_tiles[(t3, 2)])
                load_at_s_half(t3, 1, nc.sync)
                proj_piece(t3, 1, 1, ats3, wo_tiles.pop((t3, 1)),
                           out_eng=nc.scalar)
                proj_piece(t3, 2, 1, ats3, wo_tiles.pop((t3, 2)),
                           out_eng=nc.scalar)
                emit_wo_dma(t3, 0)
                proj_piece(t3, 3, 1, ats3, wo_tiles.pop((t3, 3)),
                           out_eng=nc.scalar)
                proj_piece(t3, 0, 1, ats3, wo_tiles.pop((t3, 0)),
                           out_eng=nc.scalar)
            qkvp.release()

    nc.compile()
    return nc


def _prep(inputs):
    x = np.asarray(inputs["x"], np.float32)
    freqs = np.asarray(inputs["freqs_cis"], np.float32)
    wq = np.asarray(inputs["wq"], np.float32)
    wk = np.asarray(inputs["wk"], np.float32)
    wv = np.asarray(inputs["wv"], np.float32)
    wo = np.asarray(inputs["wo"], np.float32)
    nqw = np.asarray(inputs["norm_q_w"], np.float32)
    nkw = np.asarray(inputs["norm_k_w"], np.float32)

    bf = np.float16
    xt = np.ascontiguousarray(x.reshape(BS, D).T).astype(bf)

    # de-interleave rotary pairs within each head's 128 rows
    perm = np.concatenate([np.arange(0, HD, 2), np.arange(1, HD, 2)])
    full_perm = (np.arange(H)[:, None] * HD + perm[None, :]).reshape(-1)
    wq_p = wq[full_perm]
    wk_p = wk[full_perm]

    fr = freqs.reshape(BS, HD)
    cos = np.ascontiguousarray(fr[:, :64].T)   # [64, BS]
    sin = np.ascontiguousarray(fr[:, 64:].T)
    def rot_coefs(w):
        wr = w[0::2][:, None]
        wi = w[1::2][:, None]
        plane0 = np.concatenate([wr * cos, wi * sin], axis=0)   # [128, BS]
        plane1 = np.concatenate([wr * sin, wi * cos], axis=0)
        return np.stack([plane0, plane1]).astype(bf)
    rq = rot_coefs(nqw)
    rk = rot_coefs(nkw)

    wot = np.ascontiguousarray(wo.T).astype(bf)

    in_maps = []
    for c in range(NCORES):
        r0, r1 = c * HPC * HD, (c + 1) * HPC * HD
        in_maps.append({
            "xt": xt,
            "wqt": np.ascontiguousarray(wq_p[r0:r1].T).astype(bf),
            "wkt": np.ascontiguousarray(wk_p[r0:r1].T).astype(bf),
            "wvt": np.ascontiguousarray(wv[r0:r1].T).astype(bf),
            "wot": wot,
            "rq": rq,
            "rk": rk,
        })
    return in_maps


def kernel(**inputs):
    if "nc" not in _CACHE:
        _CACHE["nc"] = _build()
    nc = _CACHE["nc"]
    in_maps = _prep(inputs)
    res = run_bass_kernel_spmd(nc, in_maps, list(range(NCORES)),
                               **_CACHE.get("run_kwargs", {}))
    _CACHE["last_result"] = res
    # core j=b*4+g owns q-tiles {4k+g, k=0..7} of batch b (row block k)
    out = np.empty((B, S, D), np.float32)
    for j in range(NCORES):
        bb, g = j // 4, j % 4
        rj = np.asarray(res.results[j]["out"]).reshape(8, 128, D)
        for k in range(8):
            t0 = (4 * k + g) * 128
            out[bb, t0:t0 + 128, :] = rj[k]
    return out

